# revision 1
# baseline (speedup 1.0000x reference)
"""Segment-mean reduction (grouped mean over sorted segment ids) on 8 trn2 cores.

Strategy (data-parallel over batch): each core handles one batch row.
out[g, :] = mean over rows s of feats with segment_ids[s] == g.

Host-side staging (inside kernel(), before upload):
  * The 1024 groups are split into 8 chunks of 128 groups. Rows of feats are
    reordered per core so that each chunk's rows are contiguous and padded to a
    multiple of 128; every 128-row tile then belongs to exactly ONE chunk, and
    the tile->chunk map is identical across cores (chunk tile counts are the
    max over cores). Pad rows point at row 0 with local id -1 (never matches).
  * feats are shipped as an fp16 hi/lo split (hi = fp16(x), lo = fp16(x - hi)),
    packed per tile as [128, 512] = [hi || lo]. fp16 streams the PE at full
    rate (fp32 is 1/4 rate); adding the hi and lo halves of the 512-wide
    matmul output recovers ~fp32 accuracy with ONE matmul per tile.

Device program per tile t (static schedule):
    onehot[s, g] = (iota[g] == sl[s])     # one tensor_scalar, DVE/GpSimd split
    psum[chunk(t)][:, 0:512] += onehot.T @ (hi || lo)   # PE, fp16 -> fp32 PSUM
and per chunk c at the end:
    sum = psum[:, :256] + psum[:, 256:]   # DVE
    out rows = sum * recip_count          # ACT copy with per-partition scale
then DMA to DRAM.

Per-core HBM traffic ~= feats (8 MB) + out (1 MB) => memory-bound.
"""

import numpy as np

import concourse.bass as bass
import concourse.bacc as bacc
import concourse.mybir as mybir
import concourse.tile as tile
from concourse.bass_utils import run_bass_kernel_spmd

F32 = mybir.dt.float32
F16 = mybir.dt.float16
P = 128  # partitions


def _host_layout(seg_all: np.ndarray, G: int):
    """Chunk-aligned row layout: shared tile->chunk map, per-core gather
    indices and aux arrays."""
    R, S = seg_all.shape
    CH = G // P

    chunk_of = seg_all // P  # [R, S]
    cnt = np.stack([np.bincount(chunk_of[r], minlength=CH) for r in range(R)])
    tiles_per_chunk = (cnt.max(axis=0) + P - 1) // P  # [CH]
    T = int(tiles_per_chunk.sum())

    chunk_of_tile = np.repeat(np.arange(CH), tiles_per_chunk)  # [T]
    first = np.full(CH, -1, np.int64)
    last = np.full(CH, -1, np.int64)
    for i, c in enumerate(chunk_of_tile):
        if first[c] < 0:
            first[c] = i
        last[c] = i

    Spad = T * P
    gather = np.zeros((R, Spad), np.int64)
    sl = np.full((R, Spad), -1.0, np.float32)  # local group id, -1 for pads
    for r in range(R):
        pos = 0
        for c in range(CH):
            rows = np.nonzero(chunk_of[r] == c)[0]
            n = len(rows)
            tc = int(tiles_per_chunk[c])
            gather[r, pos:pos + n] = rows
            sl[r, pos:pos + n] = (seg_all[r, rows] % P).astype(np.float32)
            pos += tc * P
    # aux arrays in [P, T] tile layout: column t, partition p <- padded row t*P+p
    aux_sl = np.ascontiguousarray(sl.reshape(R, T, P).transpose(0, 2, 1))
    # per-group reciprocal counts, [P, CH]: partition p, col c -> group c*P+p
    counts = np.stack(
        [np.bincount(seg_all[r], minlength=G) for r in range(R)]
    ).astype(np.float32)
    recip = (1.0 / np.maximum(counts, 1.0)).reshape(R, CH, P)
    aux_rc = np.ascontiguousarray(recip.transpose(0, 2, 1))

    return dict(T=T, CH=CH, chunk_of_tile=chunk_of_tile, first=first, last=last,
                gather=gather, aux_sl=aux_sl, aux_rc=aux_rc)


def _build_program(H: int, G: int, lay, grp: int = 8):
    T, CH = lay["T"], lay["CH"]
    chunk_of_tile = lay["chunk_of_tile"]
    first, last = lay["first"], lay["last"]
    H2 = 2 * H  # hi || lo

    nc = bacc.Bacc("TRN2", target_bir_lowering=False, debug=False, num_devices=8)
    hl_d = nc.dram_tensor("feats_hl", [T * P, H2], F16, kind="ExternalInput")
    sl_d = nc.dram_tensor("aux_sl", [P, T], F32, kind="ExternalInput")
    rc_d = nc.dram_tensor("aux_rc", [P, CH], F32, kind="ExternalInput")
    iota_d = nc.dram_tensor("iota", [P, P], F16, kind="ExternalInput")
    out_d = nc.dram_tensor("out", [G, H], F32, kind="ExternalOutput")

    with tile.TileContext(nc) as tc:
        with (
            tc.tile_pool(name="const", bufs=1) as constp,
            tc.tile_pool(name="feats", bufs=3) as fpool,
            tc.tile_pool(name="mt", bufs=8) as mtpool,
            tc.tile_pool(name="outp", bufs=2) as opool,
            tc.tile_pool(name="psum", bufs=1, space="PSUM") as pp,
        ):
            iota_t = constp.tile([P, P], F16, tag="iota")
            nc.sync.dma_start(iota_t[:], iota_d.ap())
            sl_t = constp.tile([P, T], F32, tag="sl")
            nc.sync.dma_start(sl_t[:], sl_d.ap())
            rc_t = constp.tile([P, CH], F32, tag="rc")
            nc.sync.dma_start(rc_t[:], rc_d.ap())

            psum_tiles = [
                pp.tile([P, H2], F32, tag=f"ps{c}", name=f"ps{c}") for c in range(CH)
            ]

            hl_v = hl_d.ap().rearrange("(a p) h -> p a h", p=P)

            ngrp = (T + grp - 1) // grp
            for g0 in range(ngrp):
                t0 = g0 * grp
                nt = min(grp, T - t0)
                ft = fpool.tile([P, grp, H2], F16, tag="ft")
                nc.sync.dma_start(ft[:, :nt, :], hl_v[:, t0:t0 + nt, :])
                for tt in range(nt):
                    t = t0 + tt
                    c = int(chunk_of_tile[t])
                    mt = mtpool.tile([P, P], F16, tag="mt", name="mt")
                    # onehot[s, g] = (iota[g] == sl[s]) on DVE (GpSimd is ~8x
                    # slower for this op and port-shares with DVE)
                    nc.vector.tensor_scalar(
                        mt[:],
                        iota_t[:],
                        sl_t[:, t:t + 1],
                        None,
                        mybir.AluOpType.is_equal,
                    )
                    nc.tensor.matmul(
                        psum_tiles[c][:], mt[:], ft[:, tt, :],
                        start=(t == first[c]), stop=(t == last[c]),
                    )

            for c in range(CH):
                st = opool.tile([P, H], F32, tag="st", name="st")
                ot = opool.tile([P, H], F32, tag="ot", name="ot")
                if first[c] >= 0:
                    # st = psum_hi * (1/count) on ACT (single PSUM operand)
                    nc.scalar.activation(
                        st[:], psum_tiles[c][:, :H],
                        mybir.ActivationFunctionType.Copy,
                        scale=rc_t[:, c:c + 1],
                    )
                    # ot = (psum_lo * (1/count)) + st on DVE
                    nc.vector.scalar_tensor_tensor(
                        ot[:], psum_tiles[c][:, H:], rc_t[:, c:c + 1], st[:],
                        mybir.AluOpType.mult, mybir.AluOpType.add,
                    )
                else:
                    nc.vector.memset(ot[:], 0.0)
                nc.sync.dma_start(out_d.ap()[c * P:(c + 1) * P, :], ot[:])

    nc.compile()
    return nc


def kernel(feats, segment_ids, num_groups, _trace=False):
    feats = np.ascontiguousarray(np.asarray(feats, dtype=np.float32))
    seg_all = np.ascontiguousarray(np.asarray(segment_ids, dtype=np.int32))
    G = int(num_groups)
    B, S, H = feats.shape
    assert seg_all.shape == (B, S) and B == 8 and G % P == 0

    lay = _host_layout(seg_all, G)
    nc = _build_program(H, G, lay)

    iota_arr = np.broadcast_to(
        np.arange(P, dtype=np.float16)[None, :], (P, P)
    ).copy()

    in_maps = []
    for r in range(B):
        fr = feats[r][lay["gather"][r]]  # [Spad, H] fp32, chunk-aligned
        hi = fr.astype(np.float16)
        lo = (fr - hi.astype(np.float32)).astype(np.float16)
        hl = np.concatenate([hi, lo], axis=1)  # [Spad, 2H]
        in_maps.append({
            "feats_hl": hl,
            "aux_sl": lay["aux_sl"][r],
            "aux_rc": lay["aux_rc"][r],
            "iota": iota_arr,
        })
    res = run_bass_kernel_spmd(nc, in_maps, list(range(B)), trace=_trace)
    out = np.stack([res.results[r]["out"] for r in range(B)])
    if _trace:
        return out, res
    return out



# revision 3
# speedup vs baseline: 1.3273x; 1.3273x over previous
"""Segment-mean reduction (grouped mean over sorted segment ids) on 8 trn2 cores.

Strategy (data-parallel over batch): each core handles one batch row.
out[g, :] = mean over rows s of feats with segment_ids[s] == g.

Host-side staging (inside kernel(), before upload):
  * The 1024 groups are packed per core into 8 bins of exactly 128 groups,
    balanced so each bin covers (ideally) exactly 1024 rows => T = 64 row-tiles
    of 128 with ZERO padding. Bin membership / local ids / counts are all
    per-core DATA; the program structure (tile->chunk map) is shared.
  * feats are shipped as an fp16 hi/lo split (hi = fp16(x), lo = fp16(x - hi)),
    packed PARTITION-MAJOR as [128, T*1024B] so every DMA descriptor moves
    4KB contiguous per partition (vs 1KB row-major) — keeps all 16 SDMA
    engines near line rate.
  * fp16 streams the PE at full rate; adding the hi and lo halves of the
    512-wide matmul output recovers ~fp32 accuracy with ONE matmul per tile.

Device program (static schedule), per DMA group of 4 tiles (512 KB):
    ft <- hl[:, t0*512:(t0+4)*512]          # alternating sync/scalar HWDGE ring
    onehot[s, 4, g] = (iota[g] == sl[s,t])  # ONE DVE tensor_tensor (bcast AP)
    psum[chunk(t)] += onehot_t.T @ ft_t     # PE, fp16 -> fp32 PSUM, 4 matmuls
and when tile t == last[c] (chunks finish in order, overlapped with stream):
    st = psum_hi * recip_count              # ACT copy with per-partition scale
    ot = psum_lo * recip_count + st         # DVE
    out[:, c*H:(c+1)*H] <- ot               # DMA on scalar ring
Output is partition-major [128, 8*H]; host scatters rows back to [1024, H].

Per-core HBM traffic ~= 8.39 MB feats + 1 MB out => ~26 us at 358 GB/s.
"""

import numpy as np

import concourse.bass as bass
import concourse.bacc as bacc
import concourse.mybir as mybir
import concourse.tile as tile
from concourse.bass_utils import run_bass_kernel_spmd

F32 = mybir.dt.float32
F16 = mybir.dt.float16
P = 128  # partitions
DGRP = 4  # tiles per DMA group


def _pack_bins(cnt, n_bins, slots):
    """Partition group ids into n_bins bins of exactly `slots` groups each,
    balancing row counts (sum of cnt) per bin. Returns (bins [n_bins, slots]
    int array, sums [n_bins])."""
    order = np.argsort(-cnt, kind="stable")
    bins = [[] for _ in range(n_bins)]
    sums = np.zeros(n_bins, np.int64)
    fill = np.zeros(n_bins, np.int64)
    for g in order:
        b = min((b for b in range(n_bins) if fill[b] < slots),
                key=lambda b: (sums[b], fill[b]))
        bins[b].append(int(g))
        sums[b] += cnt[g]
        fill[b] += 1
    # pairwise swap repair toward equal sums
    for _ in range(600):
        hi = int(np.argmax(sums))
        lo = int(np.argmin(sums))
        d = int(sums[hi] - sums[lo])
        if d <= 1:
            break
        ca = cnt[np.asarray(bins[hi])]
        cb = cnt[np.asarray(bins[lo])]
        delta = ca[:, None] - cb[None, :]  # effect of swapping a<->b
        good = (delta > 0) & (delta < d)
        if not good.any():
            break
        # pick swap bringing the pair closest to equal
        score = np.where(good, np.abs(d - 2 * delta), 1 << 30)
        ia, ib = np.unravel_index(np.argmin(score), score.shape)
        ga, gb = bins[hi][ia], bins[lo][ib]
        bins[hi][ia], bins[lo][ib] = gb, ga
        dd = int(cnt[ga] - cnt[gb])
        sums[hi] -= dd
        sums[lo] += dd
    return np.asarray(bins, np.int64), sums


def _host_layout(seg_all: np.ndarray, G: int):
    """Balanced-bin row layout: shared tile->chunk map, per-core gather
    indices and aux arrays."""
    R, S = seg_all.shape
    CH = G // P

    counts = np.stack([np.bincount(seg_all[r], minlength=G) for r in range(R)])
    allbins = []   # [R][CH, P] group ids
    allsums = np.zeros((R, CH), np.int64)
    for r in range(R):
        b, s = _pack_bins(counts[r], CH, P)
        allbins.append(b)
        allsums[r] = s
    # shared structure: tiles per chunk = worst core (== S//(CH*P) when balanced)
    tiles_per_chunk = (allsums.max(axis=0) + P - 1) // P  # [CH]
    T = int(tiles_per_chunk.sum())
    chunk_of_tile = np.repeat(np.arange(CH), tiles_per_chunk)  # [T]
    first = np.zeros(CH, np.int64)
    last = np.zeros(CH, np.int64)
    pos = 0
    for c in range(CH):
        first[c] = pos
        pos += int(tiles_per_chunk[c])
        last[c] = pos - 1

    Spad = T * P
    gather = np.zeros((R, Spad), np.int64)
    sl = np.full((R, Spad), -1.0, np.float16)  # local group id, -1 for pads
    aux_rc = np.zeros((R, P, CH), np.float32)
    outmap = np.zeros((R, CH, P), np.int64)
    for r in range(R):
        binid_of_group = np.zeros(G, np.int64)
        loc_of_group = np.zeros(G, np.int64)
        for c in range(CH):
            binid_of_group[allbins[r][c]] = c
            loc_of_group[allbins[r][c]] = np.arange(P)
        binid_row = binid_of_group[seg_all[r]]  # [S]
        rows_sorted = np.argsort(binid_row, kind="stable")
        row_ptr = 0
        for c in range(CH):
            n = int(allsums[r, c])
            rows = rows_sorted[row_ptr:row_ptr + n]
            row_ptr += n
            p0 = int(first[c]) * P
            gather[r, p0:p0 + n] = rows
            sl[r, p0:p0 + n] = loc_of_group[seg_all[r, rows]].astype(np.float16)
        aux_rc[r] = (1.0 / np.maximum(counts[r][allbins[r]], 1.0)).T
        outmap[r] = allbins[r]
    # sl in [P, T] tile layout: column t, partition p <- padded row t*P+p
    aux_sl = np.ascontiguousarray(
        sl.reshape(R, T, P).transpose(0, 2, 1))  # [R, P, T]
    return dict(T=T, CH=CH, chunk_of_tile=chunk_of_tile, first=first, last=last,
                gather=gather, aux_sl=aux_sl, aux_rc=aux_rc, outmap=outmap)


def _build_program(H: int, G: int, lay):
    T, CH = lay["T"], lay["CH"]
    chunk_of_tile = lay["chunk_of_tile"]
    first, last = lay["first"], lay["last"]
    H2 = 2 * H  # hi || lo

    nc = bacc.Bacc("TRN2", target_bir_lowering=False, debug=False, num_devices=8)
    hl_d = nc.dram_tensor("feats_hl", [P, T * H2], F16, kind="ExternalInput")
    sl_d = nc.dram_tensor("aux_sl", [P, T], F16, kind="ExternalInput")
    rc_d = nc.dram_tensor("aux_rc", [P, CH], F32, kind="ExternalInput")
    out_d = nc.dram_tensor("out", [P, CH * H], F32, kind="ExternalOutput")

    ngrp = (T + DGRP - 1) // DGRP

    with tile.TileContext(nc) as tc:
        with (
            tc.tile_pool(name="const", bufs=1) as constp,
            tc.tile_pool(name="feats", bufs=8) as fpool,
            tc.tile_pool(name="mt", bufs=4) as mtpool,
            tc.tile_pool(name="outp", bufs=2) as opool,
            tc.tile_pool(name="psum", bufs=1, space="PSUM") as pp,
        ):
            sl_t = constp.tile([P, T], F16, tag="sl")
            nc.sync.dma_start(sl_t[:], sl_d.ap())
            rc_t = constp.tile([P, CH], F32, tag="rc")
            nc.scalar.dma_start(rc_t[:], rc_d.ap())
            iota_t = constp.tile([P, DGRP, P], F16, tag="iota")
            nc.gpsimd.iota(iota_t[:], pattern=[[0, DGRP], [1, P]], base=0,
                           channel_multiplier=0,
                           allow_small_or_imprecise_dtypes=True)

            psum_tiles = [
                pp.tile([P, H2], F32, tag=f"ps{c}", name=f"ps{c}") for c in range(CH)
            ]

            for g0 in range(ngrp):
                t0 = g0 * DGRP
                nt = min(DGRP, T - t0)
                ft = fpool.tile([P, DGRP, H2], F16, tag="ft")
                dma_eng = nc.sync if (g0 % 2 == 0) else nc.scalar
                dma_eng.dma_start(
                    ft[:, :nt, :].rearrange("p a h -> p (a h)"),
                    hl_d.ap()[:, t0 * H2:(t0 + nt) * H2])
                mt = mtpool.tile([P, DGRP, P], F16, tag="mt", name="mt")
                # onehot[s, tt, g] = (iota[g] == sl[s, t0+tt]) on DVE, one op
                nc.vector.tensor_tensor(
                    mt[:, :nt, :],
                    iota_t[:, :nt, :],
                    sl_t[:, t0:t0 + nt].unsqueeze(2).broadcast_to((P, nt, P)),
                    mybir.AluOpType.is_equal,
                )
                for tt in range(nt):
                    t = t0 + tt
                    c = int(chunk_of_tile[t])
                    nc.tensor.matmul(
                        psum_tiles[c][:], mt[:, tt, :], ft[:, tt, :],
                        start=(t == first[c]), stop=(t == last[c]),
                    )
                    if t == last[c]:
                        st = opool.tile([P, H], F32, tag="st", name="st")
                        ot = opool.tile([P, H], F32, tag="ot", name="ot")
                        # st = psum_hi * (1/count) on ACT (single PSUM operand)
                        nc.scalar.activation(
                            st[:], psum_tiles[c][:, :H],
                            mybir.ActivationFunctionType.Copy,
                            scale=rc_t[:, c:c + 1],
                        )
                        # ot = (psum_lo * (1/count)) + st on DVE
                        nc.vector.scalar_tensor_tensor(
                            ot[:], psum_tiles[c][:, H:], rc_t[:, c:c + 1], st[:],
                            mybir.AluOpType.mult, mybir.AluOpType.add,
                        )
                        nc.scalar.dma_start(
                            out_d.ap()[:, c * H:(c + 1) * H], ot[:])

    nc.compile()
    return nc


def kernel(feats, segment_ids, num_groups, _trace=False):
    feats = np.ascontiguousarray(np.asarray(feats, dtype=np.float32))
    seg_all = np.ascontiguousarray(np.asarray(segment_ids, dtype=np.int32))
    G = int(num_groups)
    B, S, H = feats.shape
    assert seg_all.shape == (B, S) and B == 8 and G % P == 0

    lay = _host_layout(seg_all, G)
    T, CH = lay["T"], lay["CH"]
    nc = _build_program(H, G, lay)

    in_maps = []
    for r in range(B):
        fr = feats[r][lay["gather"][r]]  # [T*P, H] fp32, bin-aligned
        hi = fr.astype(np.float16)
        lo = (fr - hi.astype(np.float32)).astype(np.float16)
        hl = np.concatenate([hi, lo], axis=1)  # [T*P, 2H]
        # partition-major: [P, T*2H]; row p holds tile-column data
        hlT = np.ascontiguousarray(
            hl.reshape(T, P, 2 * H).transpose(1, 0, 2)).reshape(P, T * 2 * H)
        in_maps.append({
            "feats_hl": hlT,
            "aux_sl": np.ascontiguousarray(lay["aux_sl"][r]),
            "aux_rc": np.ascontiguousarray(lay["aux_rc"][r]),
        })
    res = run_bass_kernel_spmd(nc, in_maps, list(range(B)), trace=_trace)
    out = np.empty((B, G, H), np.float32)
    for r in range(B):
        dev = res.results[r]["out"].reshape(P, CH, H).transpose(1, 0, 2)
        out[r, lay["outmap"][r].reshape(-1)] = dev.reshape(CH * P, H)
    if _trace:
        return out, res
    return out
